# revision 14
# baseline (speedup 1.0000x reference)
"""Causal self-attention block (B=4, T=2048, C=2048, H=16, D=128) on 8 trn2 cores.

Sharding: tensor-parallel over head groups (2 groups of 8 heads) x
data-parallel over batch (4). Core (g, b) computes, for batch b and heads
[8g, 8g+8): qkv projection, causal attention, and the partial output
projection contribution attn_out[:, heads_g] @ Wproj[rows_g]. The host sums
the two partial yT per batch, adds bproj, and transposes back.

v1 design (all-bf16 matmuls, error ~4e-3 vs 2e-2 gate):
  - Everything bf16 on the PE (1 cyc/row at any width, half-cost LDWEIGHTS,
    half DMA) with f32 PSUM accumulation.
  - qkv stays in SBUF (ring over 3 heads) - no DRAM round trip.
  - Phase 1 (qkv projection) is woven at 8-matmul-run granularity between
    phase-2 attention stages of earlier heads, so the scalar engine's exp
    throughput and all DVE work hide under phase-1 PE work.
  - Softmax denominators: DVE accumulates exp blocks (two bf16 accumulators
    to cap the accumulation chain), a [128,2] ones-matmul reduces over
    partitions, reciprocal_approx_fast on [1,512], DRAM-bounce broadcast.
  - V^T -> V via DMA-engine transposes (off the PE).
  - Causal diag blocks computed at exact width (bf16 has no >=256 rule).
"""

import sys

sys.path.insert(0, "/opt/trn_rl_repo")

import numpy as np
import ml_dtypes

import concourse.bass as bass
import concourse.mybir as mybir
import concourse.tile as tile
from concourse import bacc
from concourse.bass_utils import run_bass_kernel_spmd

F32 = mybir.dt.float32
BF = mybir.dt.bfloat16
F8 = mybir.dt.float8e4
DR = mybir.MatmulPerfMode.DoubleRow
MULT = mybir.AluOpType.mult
ADD = mybir.AluOpType.add
AF = mybir.ActivationFunctionType
NPBF = ml_dtypes.bfloat16
NPF8 = ml_dtypes.float8_e4m3
W8SCALE = 128.0
TSPLIT = 512  # rows < TSPLIT run bf16; rows >= TSPLIT run fp8 e4m3

B, T, C = 4, 2048, 2048
H, D = 16, 128
G = 2  # head-group shards
HPC = H // G  # heads per core = 8
CT = C // 128  # contraction chunks = 16
NT = T // 512  # t chunks of 512 = 4
NJ = 3 * HPC  # qkv col tiles per core = 24
SCALE = 1.0 / float(np.sqrt(D))


def build_nc(phases=(1, 2, 3), reps=1):
    nc = bacc.Bacc("TRN2", target_bir_lowering=False)
    xTb = nc.dram_tensor("xTb", [128, CT, TSPLIT], BF, kind="ExternalInput")
    xT8 = nc.dram_tensor("xT8", [128, CT, T - TSPLIT], F8, kind="ExternalInput")
    wqkv = nc.dram_tensor("wqkv", [128, NJ, CT, 128], BF, kind="ExternalInput")
    wqkv8 = nc.dram_tensor("wqkv8", [128, NJ, CT, 128], F8, kind="ExternalInput")
    wproj = nc.dram_tensor("wproj", [128, CT, HPC, 128], BF, kind="ExternalInput")
    wproj8 = nc.dram_tensor("wproj8", [128, CT, HPC, 128], F8, kind="ExternalInput")
    bqkv = nc.dram_tensor("bqkv", [128, NJ], F32, kind="ExternalInput")
    tri_in = nc.dram_tensor("tri", [128, 128], BF, kind="ExternalInput")
    yT = nc.dram_tensor("yT", [C, T], BF, kind="ExternalOutput")
    yT_r = yT.rearrange("(i p) t -> p i t", p=128)

    from contextlib import ExitStack

    with tile.TileContext(nc) as tc:
        with ExitStack() as stk:
            pool = lambda *a, **k: stk.enter_context(tc.tile_pool(*a, **k))
            cst = pool(name="const", bufs=1)
            xsp = pool(name="xs", bufs=1)
            w1p = pool(name="w1", bufs=3)
            w8p = pool(name="w8", bufs=3)
            w3p = pool(name="w3", bufs=2)
            qkp = pool(name="qk", bufs=4)
            vtp = pool(name="vt", bufs=2)
            vnp = pool(name="vn", bufs=3)
            pTp = pool(name="pT", bufs=2)
            accp = pool(name="accs", bufs=4)
            rsp = pool(name="rs", bufs=2)
            rbp = pool(name="rb", bufs=2)
            otp = pool(name="ot", bufs=1)
            ysp = pool(name="ys", bufs=4)
            drp = pool(name="dramrb", bufs=3, space="DRAM")
            ps1 = pool(name="ps1", bufs=2, space="PSUM")
            pss = pool(name="pss", bufs=2, space="PSUM")
            psm = pool(name="psm", bufs=1, space="PSUM")
            pso = pool(name="pso", bufs=1, space="PSUM")
            tri = cst.tile([128, 128], BF)
            bias = cst.tile([128, NJ], F32)
            ones_f = cst.tile([128, 2], F32)
            ones = cst.tile([128, 2], BF)

            for _rep in range(reps):
                wtiles = {}

                def load_w(j):
                    w_sb = w1p.tile([128, CT, 128], BF, tag="w1", name=f"w_{j}")
                    nc.sync.dma_start(w_sb, wqkv[:, j])
                    w8_sb = w8p.tile([128, CT, 128], F8, tag="w8", name=f"w8_{j}")
                    nc.sync.dma_start(w8_sb, wqkv8[:, j])
                    wtiles[j] = (w_sb, w8_sb)

                # stage triple 0 weights interleaved with the first xs chunks
                xsb = xsp.tile([128, CT, TSPLIT], BF, tag="xsb")
                xs8 = xsp.tile([128, CT, T - TSPLIT], F8, tag="xs8")
                load_w(0)
                nc.sync.dma_start(xsb[:, 0], xTb[:, 0])
                nc.sync.dma_start(xs8[:, 0], xT8[:, 0])
                load_w(1)
                if _rep == 0:
                    nc.sync.dma_start(tri, tri_in.ap())
                    nc.sync.dma_start(bias, bqkv.ap())
                    nc.vector.memset(ones_f, 1.0 / 32.0)
                    nc.vector.tensor_copy(ones, ones_f)
                nc.sync.dma_start(xsb[:, 1], xTb[:, 1])
                nc.sync.dma_start(xs8[:, 1], xT8[:, 1])
                load_w(2)
                for cc in range(2, CT):
                    nc.sync.dma_start(xsb[:, cc], xTb[:, cc])
                    nc.sync.dma_start(xs8[:, cc], xT8[:, cc])

                qkv_dest = {}
                vn_tiles = {}
                ot_tiles = {}

                def start_tile(j):
                    p = vtp if j % 3 == 2 else qkp
                    tg = "vt" if j % 3 == 2 else "qk"
                    qkv_dest[j] = p.tile([128, T], BF, tag=tg, name=f"qkv{j}")

                def start_vn(h):
                    vn_tiles[h] = vnp.tile([128, CT, 128], BF, tag="vn", name=f"vn{h}")

                def vt_quarter_transpose(h, c):
                    """One 512-col span of V^T is ready: transpose its 4 blocks."""
                    vn = vn_tiles[h]
                    vt = qkv_dest[3 * h + 2]
                    for j in range(4 * c, 4 * c + 4):
                        nc.sync.dma_start_transpose(
                            vn[:, j], vt[:, 128 * j : 128 * (j + 1)]
                        )

                def p1_first_tile_emit():
                    """j0,j1 full + j2 chunk-0, chunk-outer, so the PE stays
                    fed while the xs load streams in."""
                    for j in (0, 1, 2):
                        start_tile(j)
                    start_vn(0)
                    pa = pss.tile([128, 2, 512], F32, tag="ps_s", name="boot_a")
                    pb = pss.tile([128, 2, 512], F32, tag="ps_s", name="boot_b")
                    c0acc = {0: pa[:, 0], 1: pa[:, 1], 2: pb[:, 0]}
                    rc1 = {0: pb[:, 1]}
                    rc1[1] = ps1.tile([128, 512], F32, tag="p1", name="boot_j1c1")
                    for u in range(8):
                        for cc in (2 * u, 2 * u + 1):
                            for j in (0, 1, 2):
                                nc.tensor.matmul(
                                    c0acc[j],
                                    lhsT=wtiles[j][0][:, cc],
                                    rhs=xsb[:, cc],
                                    start=(cc == 0),
                                    stop=(cc == CT - 1),
                                )
                        for j in (0, 1):
                            nc.tensor.matmul(
                                rc1[j],
                                lhsT=wtiles[j][1][:, 2 * u : 2 * u + 2],
                                rhs=xs8[:, 2 * u : 2 * u + 2, 0:512],
                                perf_mode=DR,
                                start=(u == 0),
                                stop=(u == 7),
                            )
                    for j in (0, 1, 2):
                        nc.scalar.activation(
                            qkv_dest[j][:, 0:512], c0acc[j], AF.Identity,
                            bias=bias[:, j : j + 1],
                        )
                    for j in (0, 1):
                        nc.scalar.activation(
                            qkv_dest[j][:, 512:1024], rc1[j], AF.Identity,
                            bias=bias[:, j : j + 1], scale=1.0 / W8SCALE,
                        )
                    vt_quarter_transpose(0, 0)
                    for j in (0, 1):
                        for c in (2, 3):
                            p1_run(j, c)

                # phase-1 work runs: chunk 0 in bf16 (two 8-matmul halves),
                # chunks 1-3 each one fp8-DoubleRow run (8 cc-pair matmuls)
                p1_state = {}

                def p1_run(j, c, half=None):
                    w_bf, w_f8 = wtiles[j]
                    if c == 0:
                        if half == 0:
                            p1_state[j] = ps1.tile(
                                [128, 512], F32, tag="p1", name=f"p1_{j}_0"
                            )
                        ps = p1_state[j]
                        for cc in range(8 * half, 8 * half + 8):
                            nc.tensor.matmul(
                                ps,
                                lhsT=w_bf[:, cc],
                                rhs=xsb[:, cc],
                                start=(cc == 0),
                                stop=(cc == CT - 1),
                            )
                        if half == 1:
                            nc.scalar.activation(
                                qkv_dest[j][:, 0:512], ps, AF.Identity,
                                bias=bias[:, j : j + 1],
                            )
                            del p1_state[j]
                        return
                    ps = ps1.tile([128, 512], F32, tag="p1", name=f"p1_{j}_{c}")
                    for u in range(8):
                        nc.tensor.matmul(
                            ps,
                            lhsT=w_f8[:, 2 * u : 2 * u + 2],
                            rhs=xs8[:, 2 * u : 2 * u + 2, 512 * (c - 1) : 512 * c],
                            perf_mode=DR,
                            start=(u == 0),
                            stop=(u == 7),
                        )
                    nc.scalar.activation(
                        qkv_dest[j][:, 512 * c : 512 * (c + 1)], ps, AF.Identity,
                        bias=bias[:, j : j + 1], scale=1.0 / W8SCALE,
                    )

                # build the full filler sequence of phase-1 emission closures
                filler = []

                def add_tile_runs(j, h, is_vt):
                    def start(jj=j, hh=h, vt_=is_vt):
                        start_tile(jj)
                        if vt_:
                            start_vn(hh)

                    filler.append(start)
                    filler.append(lambda jj=j: p1_run(jj, 0, 0))
                    filler.append(lambda jj=j: p1_run(jj, 0, 1))
                    if is_vt:
                        filler.append(lambda hh=h: vt_quarter_transpose(hh, 0))
                    for c in range(1, NT):
                        filler.append(lambda jj=j, cc_=c: p1_run(jj, cc_))
                        if is_vt:
                            filler.append(
                                lambda hh=h, cc_=c: vt_quarter_transpose(hh, cc_)
                            )

                # j=2 (vt of head 0): chunk 0 + its transposes ran in boot
                for c_ in range(1, NT):
                    filler.append(lambda cc_=c_: p1_run(2, cc_))
                    filler.append(lambda cc_=c_: vt_quarter_transpose(0, cc_))

                fill_pos = [0]

                def weave(n):
                    """emit n phase-1 filler closures if any remain"""
                    for _ in range(n):
                        if fill_pos[0] < len(filler):
                            filler[fill_pos[0]]()
                            fill_pos[0] += 1

                # ---------------- phase 2 per head, software-pipelined -------
                def p2_head(h):
                    q_sb = qkv_dest[3 * h]
                    k_sb = qkv_dest[3 * h + 1]
                    vn = vn_tiles[h]
                    pTs = {}
                    rbs = {}
                    accs = {}
                    av_queue = []  # pending AV closures

                    def s_group(c, g):
                        nblk = 4 * c + 4
                        if g == 0:
                            pTs[c] = pTp.tile([128, 16, 512], BF, tag="pT", name=f"pT_{h}_{c}")
                            acc_a = accp.tile([128, 512], BF, tag="acc", name=f"acca_{h}_{c}")
                            acc_b = (
                                accp.tile([128, 512], BF, tag="acc", name=f"accb_{h}_{c}")
                                if nblk > 8
                                else None
                            )
                            accs[c] = (acc_a, acc_b)
                        pT = pTs[c]
                        acc_a, acc_b = accs[c]
                        ps = pss.tile([128, 2, 512], F32, tag="ps_s", name=f"ss_{h}_{c}_{g}")
                        offs = []
                        for bix in range(2):
                            j = 2 * g + bix
                            v = j - 4 * c
                            off = 128 * v if v > 0 else 0
                            offs.append(off)
                            nc.tensor.matmul(
                                ps[:, bix, off:512],
                                lhsT=k_sb[:, 128 * j : 128 * (j + 1)],
                                rhs=q_sb[:, 512 * c + off : 512 * (c + 1)],
                                start=True,
                                stop=True,
                            )
                        if offs == [0, 0]:
                            nc.scalar.activation(
                                pT[:, 2 * g : 2 * g + 2, :], ps[:], AF.Exp, scale=SCALE
                            )
                        else:
                            for bix in range(2):
                                j = 2 * g + bix
                                off = offs[bix]
                                nc.scalar.activation(
                                    pT[:, j, off:512],
                                    ps[:, bix, off:512],
                                    AF.Exp,
                                    scale=SCALE,
                                )
                        for bix in range(2):
                            j = 2 * g + bix
                            v = j - 4 * c
                            off = offs[bix]
                            if v >= 0:
                                nc.gpsimd.tensor_mul(
                                    pT[:, j, off : off + 128],
                                    pT[:, j, off : off + 128],
                                    tri,
                                )
                            acc = acc_a if j < 8 else acc_b
                            if j % 8 == 0:
                                pass  # combined with j%8==1 below
                            elif j % 8 == 1:
                                nc.vector.tensor_add(
                                    acc, pT[:, j - 1, :], pT[:, j, :]
                                )
                                if off:
                                    nc.vector.tensor_copy(
                                        acc[:, 0:off], pT[:, j - 1, 0:off]
                                    )
                            else:
                                nc.vector.tensor_add(
                                    acc[:, off:512], acc[:, off:512], pT[:, j, off:512]
                                )

                    def denom(c):
                        acc_a, acc_b = accs[c]
                        ps_sum = psm.tile([2, 512], F32, tag="sum", name=f"sum_{h}_{c}")
                        nc.tensor.matmul(
                            ps_sum,
                            lhsT=ones,
                            rhs=acc_a,
                            start=True,
                            stop=(acc_b is None),
                        )
                        if acc_b is not None:
                            nc.tensor.matmul(
                                ps_sum, lhsT=ones, rhs=acc_b, start=False, stop=True
                            )
                        rs = rsp.tile([1, 512], F32, tag="rs", name=f"rs_{h}_{c}")
                        nc.vector.reciprocal_approx_fast(out=rs, in_=ps_sum[0:1, :])
                        rb = rbp.tile([128, 512], F32, tag="rb", name=f"rb_{h}_{c}")
                        nc.gpsimd.partition_broadcast(rb, rs)
                        rbs[c] = rb

                    def av_runs(c):
                        nblk = 4 * c + 4
                        pT = pTs[c]
                        state = {}

                        def run(g):
                            if g == 0:
                                state["ps"] = pso.tile([128, 512], F32, tag="o", name=f"o_{h}_{c}")
                            ps_o = state["ps"]
                            for bix in range(2):
                                j = 2 * g + bix
                                v = j - 4 * c
                                off = 128 * v if v > 0 else 0
                                nc.tensor.matmul(
                                    ps_o[:, off:512],
                                    lhsT=vn[:, j],
                                    rhs=pT[:, j, off:512],
                                    start=(j == 0),
                                    stop=(j == nblk - 1),
                                )
                            if g == nblk // 2 - 1:
                                hp, hi = h // 2, h % 2
                                key = (hp, c)
                                if key not in ot_tiles:
                                    ot_tiles[key] = otp.tile(
                                        [128, 2, 512],
                                        BF if c == 0 else F8,
                                        name=f"ot_{hp}_{c}",
                                        tag=f"ot_{hp}_{c}",
                                    )
                                nc.vector.tensor_mul(
                                    ot_tiles[key][:, hi], ps_o, rbs[c]
                                )

                        return [lambda gg=g: run(gg) for g in range(nblk // 2)]

                    for c in range(NT):
                        nblk = 4 * c + 4
                        for g in range(nblk // 2):
                            s_group(c, g)
                            # one deferred item from the previous chunk, plus
                            # a phase-1 run, between S-groups: keeps the PE fed
                            # while ACT/DVE chew on this chunk's exp/sums
                            if av_queue:
                                av_queue.pop(0)()
                            weave(1)
                        # defer this chunk's post-work into the next chunk
                        av_queue.append(lambda cc_=c: denom(cc_))
                        av_queue.extend(av_runs(c))
                    while av_queue:
                        av_queue.pop(0)()
                        weave(1)

                # ---------------- emit it all --------------------------------
                p1_first_tile_emit()
                for h in range(HPC):
                    if h + 1 < HPC:
                        for j in range(3 * (h + 1), 3 * (h + 1) + 3):
                            load_w(j)
                        for j in range(3 * (h + 1), 3 * (h + 1) + 3):
                            add_tile_runs(j, h + 1, j % 3 == 2)
                    # catch up: triples 0..h must be fully produced before
                    # head h's attention reads them
                    checkpoint = 6 + 22 * h
                    weave(max(0, checkpoint - fill_pos[0]))
                    p2_head(h)
                weave(len(filler))  # drain any left-over phase-1 work

                # ---------------- phase 3: yT = Wproj_g^T attn_outT ----------
                def load_w3(i):
                    w3 = w3p.tile([128, HPC, 128], BF, tag="w3", name=f"w3_{i}")
                    nc.sync.dma_start(w3, wproj[:, i])
                    w38 = w3p.tile([128, HPC, 128], F8, tag="w38", name=f"w38_{i}")
                    nc.sync.dma_start(w38, wproj8[:, i])
                    return w3, w38

                w3_cur = load_w3(0)
                for i in range(CT):
                    w3_nxt = load_w3(i + 1) if i + 1 < CT else None
                    for c in range(NT):
                        ps_y = ps1.tile([128, 512], F32, tag="p1", name=f"y_{i}_{c}")
                        if c == 0:
                            for hh in range(HPC):
                                nc.tensor.matmul(
                                    ps_y,
                                    lhsT=w3_cur[0][:, hh],
                                    rhs=ot_tiles[(hh // 2, 0)][:, hh % 2],
                                    start=(hh == 0),
                                    stop=(hh == HPC - 1),
                                )
                        else:
                            for u in range(HPC // 2):
                                nc.tensor.matmul(
                                    ps_y,
                                    lhsT=w3_cur[1][:, 2 * u : 2 * u + 2],
                                    rhs=ot_tiles[(u, c)][:],
                                    perf_mode=DR,
                                    start=(u == 0),
                                    stop=(u == HPC // 2 - 1),
                                )
                        ys = ysp.tile([128, 512], BF, tag="ys", name=f"ys_{i}_{c}")
                        descale = 1.0 / 32.0 if c == 0 else 1.0 / (32.0 * W8SCALE)
                        nc.vector.tensor_scalar_mul(ys, ps_y, descale)
                        nc.sync.dma_start(yT_r[:, i, 512 * c : 512 * (c + 1)], ys)
                    w3_cur = w3_nxt

    nc.compile()
    return nc


_NC_CACHE = None


def _get_nc():
    global _NC_CACHE
    if _NC_CACHE is None:
        _NC_CACHE = build_nc()
    return _NC_CACHE


def _prep_inputs(x, Wqkv, bqkv, Wproj):
    """Host-side shard + pre-tile + bf16 cast. Returns list of 8 in_maps,
    core index = g * B + b."""
    x = np.asarray(x, dtype=np.float32)
    Wqkv = np.asarray(Wqkv, dtype=np.float32)
    Wproj = np.asarray(Wproj, dtype=np.float32)
    bqkv = np.asarray(bqkv, dtype=np.float32)

    p = np.arange(128)[:, None]
    f = np.arange(128)[None, :]
    tri = (f >= p).astype(NPBF)
    tri = np.ascontiguousarray(tri)

    # xT tiles per batch: [128, CT, T] with [p, o, t] = x[b, t, o*128+p];
    # t < TSPLIT shipped in bf16, the rest in fp8 e4m3
    xTb_b, xT8_b = [], []
    for b in range(B):
        xt = x[b].T  # [C, T] f32
        xt_t = xt.reshape(CT, 128, T).transpose(1, 0, 2)  # [128, CT, T]
        xTb_b.append(np.ascontiguousarray(xt_t[:, :, :TSPLIT].astype(NPBF)))
        xT8_b.append(
            np.ascontiguousarray(
                np.clip(xt_t[:, :, TSPLIT:], -240, 240).astype(NPF8)
            )
        )

    in_maps = [None] * (G * B)
    for g in range(G):
        # col-tile j = 3h + kind (kind: 0=q, 1=k, 2=v)
        cols = np.concatenate(
            [
                np.arange(kind * C + g * 1024 + h * 128, kind * C + g * 1024 + (h + 1) * 128)
                for h in range(HPC)
                for kind in range(3)
            ]
        )
        wg = Wqkv[:, cols]  # [C, 3072] j-major
        wg_r = wg.reshape(CT, 128, NJ, 128).transpose(1, 2, 0, 3)
        wg_t = np.ascontiguousarray(wg_r.astype(NPBF))
        wg_8 = np.ascontiguousarray(
            np.clip(wg_r * W8SCALE, -240, 240).astype(NPF8)
        )
        bg_t = np.ascontiguousarray(bqkv[cols].reshape(NJ, 128).T)  # [128, 24] f32
        wp = Wproj[g * 1024 : (g + 1) * 1024, :]  # [1024, C]
        wp_r = wp.reshape(HPC, 128, CT, 128).transpose(1, 2, 0, 3)
        wp_t = np.ascontiguousarray(wp_r.astype(NPBF))
        wp_8 = np.ascontiguousarray(
            np.clip(wp_r * W8SCALE, -240, 240).astype(NPF8)
        )
        for b in range(B):
            in_maps[g * B + b] = dict(
                xTb=xTb_b[b],
                xT8=xT8_b[b],
                wqkv=wg_t,
                wqkv8=wg_8,
                wproj=wp_t,
                wproj8=wp_8,
                bqkv=bg_t,
                tri=tri,
            )
    return in_maps


def kernel(x, Wqkv, bqkv, Wproj, bproj):
    x = np.asarray(x)
    nc = _get_nc()
    in_maps = _prep_inputs(x, Wqkv, bqkv, Wproj)
    res = run_bass_kernel_spmd(nc, in_maps, core_ids=list(range(G * B)))
    y = np.empty((B, T, C), dtype=np.float32)
    bp = np.asarray(bproj, dtype=np.float32)
    for b in range(B):
        acc = res.results[b]["yT"].astype(np.float32)
        for g in range(1, G):
            acc = acc + res.results[g * B + b]["yT"].astype(np.float32)
        y[b] = acc.T + bp[None, :]
    return y


# revision 15
# speedup vs baseline: 1.6054x; 1.6054x over previous
"""Causal self-attention block (B=4, T=2048, C=2048, H=16, D=128) on 8 trn2 cores.

Sharding: tensor-parallel over head groups (2 groups of 8 heads) x
data-parallel over batch (4). Core (g, b) computes, for batch b and heads
[8g, 8g+8): qkv projection, causal attention, and the partial output
projection contribution attn_out[:, heads_g] @ Wproj[rows_g]. The host sums
the two partial yT per batch, adds bproj, and transposes back.

v1 design (all-bf16 matmuls, error ~4e-3 vs 2e-2 gate):
  - Everything bf16 on the PE (1 cyc/row at any width, half-cost LDWEIGHTS,
    half DMA) with f32 PSUM accumulation.
  - qkv stays in SBUF (ring over 3 heads) - no DRAM round trip.
  - Phase 1 (qkv projection) is woven at 8-matmul-run granularity between
    phase-2 attention stages of earlier heads, so the scalar engine's exp
    throughput and all DVE work hide under phase-1 PE work.
  - Softmax denominators: DVE accumulates exp blocks (two bf16 accumulators
    to cap the accumulation chain), a [128,2] ones-matmul reduces over
    partitions, reciprocal_approx_fast on [1,512], DRAM-bounce broadcast.
  - V^T -> V via DMA-engine transposes (off the PE).
  - Causal diag blocks computed at exact width (bf16 has no >=256 rule).
"""

import sys

sys.path.insert(0, "/opt/trn_rl_repo")

import numpy as np
import ml_dtypes

import concourse.bass as bass
import concourse.mybir as mybir
import concourse.tile as tile
from concourse import bacc
from concourse.bass_utils import run_bass_kernel_spmd

F32 = mybir.dt.float32
BF = mybir.dt.bfloat16
F8 = mybir.dt.float8e4
DR = mybir.MatmulPerfMode.DoubleRow
MULT = mybir.AluOpType.mult
ADD = mybir.AluOpType.add
AF = mybir.ActivationFunctionType
NPBF = ml_dtypes.bfloat16
NPF8 = ml_dtypes.float8_e4m3
W8SCALE = 128.0
TSPLIT = 512  # rows < TSPLIT run bf16; rows >= TSPLIT run fp8 e4m3

B, T, C = 4, 2048, 2048
H, D = 16, 128
G = 2  # head-group shards
HPC = H // G  # heads per core = 8
CT = C // 128  # contraction chunks = 16
NT = T // 512  # t chunks of 512 = 4
NJ = 3 * HPC  # qkv col tiles per core = 24
SCALE = 1.0 / float(np.sqrt(D))


def build_nc(phases=(1, 2, 3), reps=1):
    nc = bacc.Bacc("TRN2", target_bir_lowering=False)
    xTb = nc.dram_tensor("xTb", [128, CT, TSPLIT], BF, kind="ExternalInput")
    xT8 = nc.dram_tensor("xT8", [128, CT, T - TSPLIT], F8, kind="ExternalInput")
    wqkv = nc.dram_tensor("wqkv", [128, NJ, CT, 128], BF, kind="ExternalInput")
    wqkv8 = nc.dram_tensor("wqkv8", [128, NJ, CT, 128], F8, kind="ExternalInput")
    wproj = nc.dram_tensor("wproj", [128, CT, HPC, 128], BF, kind="ExternalInput")
    wproj8 = nc.dram_tensor("wproj8", [128, CT, HPC, 128], F8, kind="ExternalInput")
    bqkv = nc.dram_tensor("bqkv", [128, NJ], F32, kind="ExternalInput")
    tri_in = nc.dram_tensor("tri", [128, 128], BF, kind="ExternalInput")
    yT = nc.dram_tensor("yT", [C, T], BF, kind="ExternalOutput")
    yT_r = yT.rearrange("(i p) t -> p i t", p=128)

    from contextlib import ExitStack

    with tile.TileContext(nc) as tc:
        with ExitStack() as stk:
            pool = lambda *a, **k: stk.enter_context(tc.tile_pool(*a, **k))
            cst = pool(name="const", bufs=1)
            xsp = pool(name="xs", bufs=1)
            w1p = pool(name="w1", bufs=3)
            w8p = pool(name="w8", bufs=3)
            w3p = pool(name="w3", bufs=2)
            qkp = pool(name="qk", bufs=4)
            vtp = pool(name="vt", bufs=2)
            vnp = pool(name="vn", bufs=3)
            pTp = pool(name="pT", bufs=2)
            accp = pool(name="accs", bufs=4)
            rsp = pool(name="rs", bufs=2)
            rbp = pool(name="rb", bufs=2)
            otp = pool(name="ot", bufs=1)
            ysp = pool(name="ys", bufs=4)
            drp = pool(name="dramrb", bufs=3, space="DRAM")
            ps1 = pool(name="ps1", bufs=2, space="PSUM")
            pss = pool(name="pss", bufs=2, space="PSUM")
            psm = pool(name="psm", bufs=1, space="PSUM")
            pso = pool(name="pso", bufs=1, space="PSUM")
            tri = cst.tile([128, 128], BF)
            bias = cst.tile([128, NJ], F32)
            ones_f = cst.tile([128, 2], F32)
            ones = cst.tile([128, 2], BF)

            for _rep in range(reps):
                wtiles = {}

                def load_w(j):
                    w_sb = w1p.tile([128, CT, 128], BF, tag="w1", name=f"w_{j}")
                    nc.sync.dma_start(w_sb, wqkv[:, j])
                    w8_sb = w8p.tile([128, CT, 128], F8, tag="w8", name=f"w8_{j}")
                    nc.sync.dma_start(w8_sb, wqkv8[:, j])
                    wtiles[j] = (w_sb, w8_sb)

                # stage triple 0 weights interleaved with the first xs chunks
                xsb = xsp.tile([128, CT, TSPLIT], BF, tag="xsb")
                xs8 = xsp.tile([128, CT, T - TSPLIT], F8, tag="xs8")
                load_w(0)
                nc.sync.dma_start(xsb[:, 0], xTb[:, 0])
                nc.sync.dma_start(xs8[:, 0], xT8[:, 0])
                load_w(1)
                if _rep == 0:
                    nc.sync.dma_start(tri, tri_in.ap())
                    nc.sync.dma_start(bias, bqkv.ap())
                    nc.vector.memset(ones_f, 1.0 / 32.0)
                    nc.vector.tensor_copy(ones, ones_f)
                nc.sync.dma_start(xsb[:, 1], xTb[:, 1])
                nc.sync.dma_start(xs8[:, 1], xT8[:, 1])
                load_w(2)
                for cc in range(2, CT):
                    nc.sync.dma_start(xsb[:, cc], xTb[:, cc])
                    nc.sync.dma_start(xs8[:, cc], xT8[:, cc])

                qkv_dest = {}
                vn_tiles = {}
                ot_tiles = {}

                def start_tile(j):
                    p = vtp if j % 3 == 2 else qkp
                    tg = "vt" if j % 3 == 2 else "qk"
                    qkv_dest[j] = p.tile([128, T], BF, tag=tg, name=f"qkv{j}")

                def start_vn(h):
                    vn_tiles[h] = vnp.tile([128, CT, 128], BF, tag="vn", name=f"vn{h}")

                def vt_quarter_transpose(h, c):
                    """One 512-col span of V^T is ready: transpose its 4 blocks."""
                    vn = vn_tiles[h]
                    vt = qkv_dest[3 * h + 2]
                    for j in range(4 * c, 4 * c + 4):
                        nc.sync.dma_start_transpose(
                            vn[:, j], vt[:, 128 * j : 128 * (j + 1)]
                        )

                def p1_first_tile_emit():
                    """j0,j1 full + j2 chunk-0, chunk-outer, so the PE stays
                    fed while the xs load streams in."""
                    for j in (0, 1, 2):
                        start_tile(j)
                    start_vn(0)
                    pa = pss.tile([128, 2, 512], F32, tag="ps_s", name="boot_a")
                    pb = pss.tile([128, 2, 512], F32, tag="ps_s", name="boot_b")
                    c0acc = {0: pa[:, 0], 1: pa[:, 1], 2: pb[:, 0]}
                    rc1 = {0: pb[:, 1]}
                    rc1[1] = ps1.tile([128, 512], F32, tag="p1", name="boot_j1c1")
                    for u in range(8):
                        for cc in (2 * u, 2 * u + 1):
                            for j in (0, 1, 2):
                                nc.tensor.matmul(
                                    c0acc[j],
                                    lhsT=wtiles[j][0][:, cc],
                                    rhs=xsb[:, cc],
                                    start=(cc == 0),
                                    stop=(cc == CT - 1),
                                )
                        for j in (0, 1):
                            nc.tensor.matmul(
                                rc1[j],
                                lhsT=wtiles[j][1][:, 2 * u : 2 * u + 2],
                                rhs=xs8[:, 2 * u : 2 * u + 2, 0:512],
                                perf_mode=DR,
                                start=(u == 0),
                                stop=(u == 7),
                            )
                    for j in (0, 1, 2):
                        nc.scalar.activation(
                            qkv_dest[j][:, 0:512], c0acc[j], AF.Identity,
                            bias=bias[:, j : j + 1],
                        )
                    for j in (0, 1):
                        nc.scalar.activation(
                            qkv_dest[j][:, 512:1024], rc1[j], AF.Identity,
                            bias=bias[:, j : j + 1], scale=1.0 / W8SCALE,
                        )
                    vt_quarter_transpose(0, 0)
                    for j in (0, 1):
                        for c in (2, 3):
                            p1_run(j, c)

                # phase-1 work runs: chunk 0 in bf16 (two 8-matmul halves),
                # chunks 1-3 each one fp8-DoubleRow run (8 cc-pair matmuls)
                p1_state = {}

                def p1_run(j, c, half=None):
                    w_bf, w_f8 = wtiles[j]
                    if c == 0:
                        if half == 0:
                            p1_state[j] = ps1.tile(
                                [128, 512], F32, tag="p1", name=f"p1_{j}_0"
                            )
                        ps = p1_state[j]
                        for cc in range(8 * half, 8 * half + 8):
                            nc.tensor.matmul(
                                ps,
                                lhsT=w_bf[:, cc],
                                rhs=xsb[:, cc],
                                start=(cc == 0),
                                stop=(cc == CT - 1),
                            )
                        if half == 1:
                            nc.scalar.activation(
                                qkv_dest[j][:, 0:512], ps, AF.Identity,
                                bias=bias[:, j : j + 1],
                            )
                            del p1_state[j]
                        return
                    ps = ps1.tile([128, 512], F32, tag="p1", name=f"p1_{j}_{c}")
                    for u in range(8):
                        nc.tensor.matmul(
                            ps,
                            lhsT=w_f8[:, 2 * u : 2 * u + 2],
                            rhs=xs8[:, 2 * u : 2 * u + 2, 512 * (c - 1) : 512 * c],
                            perf_mode=DR,
                            start=(u == 0),
                            stop=(u == 7),
                        )
                    nc.scalar.activation(
                        qkv_dest[j][:, 512 * c : 512 * (c + 1)], ps, AF.Identity,
                        bias=bias[:, j : j + 1], scale=1.0 / W8SCALE,
                    )

                # build the full filler sequence of phase-1 emission closures
                filler = []

                def add_tile_runs(j, h, is_vt):
                    def start(jj=j, hh=h, vt_=is_vt):
                        start_tile(jj)
                        if vt_:
                            start_vn(hh)

                    filler.append(start)
                    filler.append(lambda jj=j: p1_run(jj, 0, 0))
                    filler.append(lambda jj=j: p1_run(jj, 0, 1))
                    if is_vt:
                        filler.append(lambda hh=h: vt_quarter_transpose(hh, 0))
                    for c in range(1, NT):
                        filler.append(lambda jj=j, cc_=c: p1_run(jj, cc_))
                        if is_vt:
                            filler.append(
                                lambda hh=h, cc_=c: vt_quarter_transpose(hh, cc_)
                            )

                # j=2 (vt of head 0): chunk 0 + its transposes ran in boot
                for c_ in range(1, NT):
                    filler.append(lambda cc_=c_: p1_run(2, cc_))
                    filler.append(lambda cc_=c_: vt_quarter_transpose(0, cc_))

                fill_pos = [0]

                def weave(n):
                    """emit n phase-1 filler closures if any remain"""
                    for _ in range(n):
                        if fill_pos[0] < len(filler):
                            filler[fill_pos[0]]()
                            fill_pos[0] += 1

                # ---------------- phase 2 per head, software-pipelined -------
                def p2_head(h):
                    q_sb = qkv_dest[3 * h]
                    k_sb = qkv_dest[3 * h + 1]
                    vn = vn_tiles[h]
                    pTs = {}
                    rbs = {}
                    accs = {}
                    av_queue = []  # pending AV closures

                    def s_group(c, g):
                        nblk = 4 * c + 4
                        if g == 0:
                            pTs[c] = pTp.tile([128, 16, 512], BF, tag="pT", name=f"pT_{h}_{c}")
                            acc_a = accp.tile([128, 512], BF, tag="acc", name=f"acca_{h}_{c}")
                            acc_b = (
                                accp.tile([128, 512], BF, tag="acc", name=f"accb_{h}_{c}")
                                if nblk > 8
                                else None
                            )
                            accs[c] = (acc_a, acc_b)
                        pT = pTs[c]
                        acc_a, acc_b = accs[c]
                        ps = pss.tile([128, 2, 512], F32, tag="ps_s", name=f"ss_{h}_{c}_{g}")
                        offs = []
                        for bix in range(2):
                            j = 2 * g + bix
                            v = j - 4 * c
                            off = 128 * v if v > 0 else 0
                            offs.append(off)
                            nc.tensor.matmul(
                                ps[:, bix, off:512],
                                lhsT=k_sb[:, 128 * j : 128 * (j + 1)],
                                rhs=q_sb[:, 512 * c + off : 512 * (c + 1)],
                                start=True,
                                stop=True,
                            )
                        if offs == [0, 0]:
                            nc.scalar.activation(
                                pT[:, 2 * g : 2 * g + 2, :], ps[:], AF.Exp, scale=SCALE
                            )
                        else:
                            for bix in range(2):
                                j = 2 * g + bix
                                off = offs[bix]
                                nc.scalar.activation(
                                    pT[:, j, off:512],
                                    ps[:, bix, off:512],
                                    AF.Exp,
                                    scale=SCALE,
                                )
                        for bix in range(2):
                            j = 2 * g + bix
                            v = j - 4 * c
                            off = offs[bix]
                            if v >= 0:
                                nc.vector.tensor_mul(
                                    pT[:, j, off : off + 128],
                                    pT[:, j, off : off + 128],
                                    tri,
                                )
                            acc = acc_a if j < 8 else acc_b
                            if j % 8 == 0:
                                pass  # combined with j%8==1 below
                            elif j % 8 == 1:
                                nc.vector.tensor_add(
                                    acc, pT[:, j - 1, :], pT[:, j, :]
                                )
                                if off:
                                    nc.vector.tensor_copy(
                                        acc[:, 0:off], pT[:, j - 1, 0:off]
                                    )
                            else:
                                nc.vector.tensor_add(
                                    acc[:, off:512], acc[:, off:512], pT[:, j, off:512]
                                )

                    def denom(c):
                        acc_a, acc_b = accs[c]
                        ps_sum = psm.tile([2, 512], F32, tag="sum", name=f"sum_{h}_{c}")
                        nc.tensor.matmul(
                            ps_sum,
                            lhsT=ones,
                            rhs=acc_a,
                            start=True,
                            stop=(acc_b is None),
                        )
                        if acc_b is not None:
                            nc.tensor.matmul(
                                ps_sum, lhsT=ones, rhs=acc_b, start=False, stop=True
                            )
                        rs = rsp.tile([1, 512], F32, tag="rs", name=f"rs_{h}_{c}")
                        nc.vector.reciprocal_approx_fast(out=rs, in_=ps_sum[0:1, :])
                        rb = rbp.tile([128, 512], F32, tag="rb", name=f"rb_{h}_{c}")
                        nc.gpsimd.partition_broadcast(rb, rs)
                        rbs[c] = rb

                    def av_runs(c):
                        nblk = 4 * c + 4
                        pT = pTs[c]
                        state = {}

                        def run(g):
                            if g == 0:
                                state["ps"] = pso.tile([128, 512], F32, tag="o", name=f"o_{h}_{c}")
                            ps_o = state["ps"]
                            for bix in range(2):
                                j = 2 * g + bix
                                v = j - 4 * c
                                off = 128 * v if v > 0 else 0
                                nc.tensor.matmul(
                                    ps_o[:, off:512],
                                    lhsT=vn[:, j],
                                    rhs=pT[:, j, off:512],
                                    start=(j == 0),
                                    stop=(j == nblk - 1),
                                )
                            if g == nblk // 2 - 1:
                                hp, hi = h // 2, h % 2
                                key = (hp, c)
                                if key not in ot_tiles:
                                    ot_tiles[key] = otp.tile(
                                        [128, 2, 512],
                                        BF if c == 0 else F8,
                                        name=f"ot_{hp}_{c}",
                                        tag=f"ot_{hp}_{c}",
                                    )
                                nc.vector.tensor_mul(
                                    ot_tiles[key][:, hi], ps_o, rbs[c]
                                )

                        return [lambda gg=g: run(gg) for g in range(nblk // 2)]

                    for c in range(NT):
                        nblk = 4 * c + 4
                        for g in range(nblk // 2):
                            s_group(c, g)
                            # one deferred item from the previous chunk, plus
                            # a phase-1 run, between S-groups: keeps the PE fed
                            # while ACT/DVE chew on this chunk's exp/sums
                            if av_queue:
                                av_queue.pop(0)()
                            weave(1)
                        # defer this chunk's post-work into the next chunk
                        av_queue.append(lambda cc_=c: denom(cc_))
                        av_queue.extend(av_runs(c))
                    while av_queue:
                        av_queue.pop(0)()
                        weave(1)

                # ---------------- emit it all --------------------------------
                p1_first_tile_emit()
                for h in range(HPC):
                    if h + 1 < HPC:
                        for j in range(3 * (h + 1), 3 * (h + 1) + 3):
                            load_w(j)
                        for j in range(3 * (h + 1), 3 * (h + 1) + 3):
                            add_tile_runs(j, h + 1, j % 3 == 2)
                    # catch up: triples 0..h must be fully produced before
                    # head h's attention reads them
                    checkpoint = 6 + 22 * h
                    weave(max(0, checkpoint - fill_pos[0]))
                    p2_head(h)
                weave(len(filler))  # drain any left-over phase-1 work

                # ---------------- phase 3: yT = Wproj_g^T attn_outT ----------
                def load_w3(i):
                    w3 = w3p.tile([128, HPC, 128], BF, tag="w3", name=f"w3_{i}")
                    nc.sync.dma_start(w3, wproj[:, i])
                    w38 = w3p.tile([128, HPC, 128], F8, tag="w38", name=f"w38_{i}")
                    nc.sync.dma_start(w38, wproj8[:, i])
                    return w3, w38

                w3_cur = load_w3(0)
                for i in range(CT):
                    w3_nxt = load_w3(i + 1) if i + 1 < CT else None
                    for c in range(NT):
                        ps_y = ps1.tile([128, 512], F32, tag="p1", name=f"y_{i}_{c}")
                        if c == 0:
                            for hh in range(HPC):
                                nc.tensor.matmul(
                                    ps_y,
                                    lhsT=w3_cur[0][:, hh],
                                    rhs=ot_tiles[(hh // 2, 0)][:, hh % 2],
                                    start=(hh == 0),
                                    stop=(hh == HPC - 1),
                                )
                        else:
                            for u in range(HPC // 2):
                                nc.tensor.matmul(
                                    ps_y,
                                    lhsT=w3_cur[1][:, 2 * u : 2 * u + 2],
                                    rhs=ot_tiles[(u, c)][:],
                                    perf_mode=DR,
                                    start=(u == 0),
                                    stop=(u == HPC // 2 - 1),
                                )
                        ys = ysp.tile([128, 512], BF, tag="ys", name=f"ys_{i}_{c}")
                        descale = 1.0 / 32.0 if c == 0 else 1.0 / (32.0 * W8SCALE)
                        nc.vector.tensor_scalar_mul(ys, ps_y, descale)
                        nc.sync.dma_start(yT_r[:, i, 512 * c : 512 * (c + 1)], ys)
                    w3_cur = w3_nxt

    nc.compile()
    return nc


_NC_CACHE = None


def _get_nc():
    global _NC_CACHE
    if _NC_CACHE is None:
        _NC_CACHE = build_nc()
    return _NC_CACHE


def _prep_inputs(x, Wqkv, bqkv, Wproj):
    """Host-side shard + pre-tile + bf16 cast. Returns list of 8 in_maps,
    core index = g * B + b."""
    x = np.asarray(x, dtype=np.float32)
    Wqkv = np.asarray(Wqkv, dtype=np.float32)
    Wproj = np.asarray(Wproj, dtype=np.float32)
    bqkv = np.asarray(bqkv, dtype=np.float32)

    p = np.arange(128)[:, None]
    f = np.arange(128)[None, :]
    tri = (f >= p).astype(NPBF)
    tri = np.ascontiguousarray(tri)

    # xT tiles per batch: [128, CT, T] with [p, o, t] = x[b, t, o*128+p];
    # t < TSPLIT shipped in bf16, the rest in fp8 e4m3
    xTb_b, xT8_b = [], []
    for b in range(B):
        xt = x[b].T  # [C, T] f32
        xt_t = xt.reshape(CT, 128, T).transpose(1, 0, 2)  # [128, CT, T]
        xTb_b.append(np.ascontiguousarray(xt_t[:, :, :TSPLIT].astype(NPBF)))
        xT8_b.append(
            np.ascontiguousarray(
                np.clip(xt_t[:, :, TSPLIT:], -240, 240).astype(NPF8)
            )
        )

    in_maps = [None] * (G * B)
    for g in range(G):
        # col-tile j = 3h + kind (kind: 0=q, 1=k, 2=v)
        cols = np.concatenate(
            [
                np.arange(kind * C + g * 1024 + h * 128, kind * C + g * 1024 + (h + 1) * 128)
                for h in range(HPC)
                for kind in range(3)
            ]
        )
        wg = Wqkv[:, cols]  # [C, 3072] j-major
        wg_r = wg.reshape(CT, 128, NJ, 128).transpose(1, 2, 0, 3)
        wg_t = np.ascontiguousarray(wg_r.astype(NPBF))
        wg_8 = np.ascontiguousarray(
            np.clip(wg_r * W8SCALE, -240, 240).astype(NPF8)
        )
        bg_t = np.ascontiguousarray(bqkv[cols].reshape(NJ, 128).T)  # [128, 24] f32
        wp = Wproj[g * 1024 : (g + 1) * 1024, :]  # [1024, C]
        wp_r = wp.reshape(HPC, 128, CT, 128).transpose(1, 2, 0, 3)
        wp_t = np.ascontiguousarray(wp_r.astype(NPBF))
        wp_8 = np.ascontiguousarray(
            np.clip(wp_r * W8SCALE, -240, 240).astype(NPF8)
        )
        for b in range(B):
            in_maps[g * B + b] = dict(
                xTb=xTb_b[b],
                xT8=xT8_b[b],
                wqkv=wg_t,
                wqkv8=wg_8,
                wproj=wp_t,
                wproj8=wp_8,
                bqkv=bg_t,
                tri=tri,
            )
    return in_maps


def kernel(x, Wqkv, bqkv, Wproj, bproj):
    x = np.asarray(x)
    nc = _get_nc()
    in_maps = _prep_inputs(x, Wqkv, bqkv, Wproj)
    res = run_bass_kernel_spmd(nc, in_maps, core_ids=list(range(G * B)))
    y = np.empty((B, T, C), dtype=np.float32)
    bp = np.asarray(bproj, dtype=np.float32)
    for b in range(B):
        acc = res.results[b]["yT"].astype(np.float32)
        for g in range(1, G):
            acc = acc + res.results[g * B + b]["yT"].astype(np.float32)
        y[b] = acc.T + bp[None, :]
    return y
